# revision 57
# baseline (speedup 1.0000x reference)
"""GATv2 layer kernel for Trainium2 — 8 NeuronCores, SPMD row-sharded.

Math (reference):
    h = x @ W
    s1 = h @ a[:F];  s2 = h @ a[F:]
    e  = leaky_relu(s1[:,None] + s2[None,:], 0.2)
    e  = where(adj > 0, e, -9e15)
    att = softmax(e, axis=1)
    out = elu(att @ h)

Kernel strategy (per core, rows of the output sharded across 8 cores):
  - Inputs are fed pre-transposed from the host: xT = x.T (fp16) and
    adjT = adj[own_rows].T (fp16, {0,1}); both have the j-dimension
    permuted so this core's own rows come first (the permutation is
    consistent across xT columns / h chunks / adjT rows, and the j-sum
    of the attention matmul is permutation invariant). With j on
    partitions everything downstream needs no on-device transposes.
  - h chunks [128 j, 128 f] from xT-chunk @ [W | W a1 | W a2]; the two
    extra columns give s1 (own rows = first 8 chunks) and s2 per chunk.
  - E^T[j, i] = exp(lrelu(s1_i + s2_j)) computed per j-chunk either on
    ACT (Prelu with per-partition s2 bias over a broadcast s1 row, then
    Exp) or via the exact rank-1 identity
        E = max(exp(s1)exp(s2), exp(s1/5)exp(s2/5))
    on DVE tensor_scalar (4x mode) + a tensor_tensor max (DVE or Pool).
  - P^T = E^T * adjT with a 2-byte tensor_tensor multiply (DVE or Pool);
    the masked-softmax -9e15 becomes an exact multiplicative {0,1} mask
    (softmax row scale cancels).
  - attention: h'^T[f, i] += h_c^T.T @ P^T_c on PE; row-sums are
    ones-vector matmuls accumulated alongside in PSUM.
  - final: transpose h'^T back, out = elu(h' / rowsum), stream to DRAM.
"""

import sys

if "/opt/trn_rl_repo" not in sys.path:
    sys.path.insert(0, "/opt/trn_rl_repo")

from contextlib import ExitStack

import numpy as np

import concourse.bass as bass
import concourse.tile as tile
from concourse import bacc, mybir
from concourse.masks import make_identity

F32 = mybir.dt.float32
F16 = mybir.dt.float16
AF = mybir.ActivationFunctionType
OP = mybir.AluOpType

N_FULL = 8192
F_IN = 256
F_OUT = 128
N_CORES = 8
NEG_SLOPE = 0.2


def _spread(k, total, count):
    """True for `count` of the `total` indices, evenly spread."""
    return (k * count) // total != ((k + 1) * count) // total


def build_gat(
    n=N_FULL,
    rows=N_FULL // N_CORES,
    f_in=F_IN,
    f_out=F_OUT,
    n_act=32,          # chunks whose E runs on ACT (prelu+exp)
    n_maxpool=0,       # tt-max on Pool is not ISA-valid; keep 0
    n_maskpool=32,     # chunks whose mask-mult runs on Pool
    adjb=4,            # adjT chunks per batched DMA
    adj_bufs=8,
    e_bufs=12,
    p_bufs=10,
    xg=8,              # chunks per xT load group
    h_copy_cycle="vva",  # engines for hps->h_sb copies: v=DVE a=ACT
    adj_dma="sync",    # issue adjT DMAs from: act|sync
    hpsp_bufs=3,
    lag=10,
):
    KC = f_in // 128
    NCH = n // 128            # j chunks
    SUB = rows // 128         # output subtiles
    FO2 = f_out + 2
    IH = rows // 512          # rowsum/acc PSUM halves

    nc = bacc.Bacc(
        "TRN2",
        target_bir_lowering=False,
        debug=False,
        enable_asserts=False,
        num_devices=1,
    )
    xt_ap = nc.dram_tensor("xt", [f_in, n], F16, kind="ExternalInput").ap()
    adjt_ap = nc.dram_tensor("adjt", [n, rows], F16, kind="ExternalInput").ap()
    w_ap = nc.dram_tensor("w", [f_in, f_out], F32, kind="ExternalInput").ap()
    a_ap = nc.dram_tensor("a", [2 * f_out, 1], F32, kind="ExternalInput").ap()
    out_ap = nc.dram_tensor("out", [rows, f_out], F32, kind="ExternalOutput").ap()
    bvec = nc.dram_tensor("bvec", [3 * rows], F16, kind="Internal").ap()
    rsd = nc.dram_tensor("rsd", [2 * rows], F32, kind="Internal").ap()

    def dram3(ap, off, dims):
        return bass.AP(tensor=ap.tensor, offset=ap.offset + off, ap=dims)

    # E-route / mask / max engine per chunk, evenly interleaved.
    # Pool masks go preferentially to the DVE-route chunks so no chunk's
    # E+mask chain stacks entirely on one engine.
    act_route = [_spread(k, NCH, n_act) for k in range(NCH)]
    dve_chunks = [k for k in range(NCH) if not act_route[k]]
    maxpool = set()
    for idx, k in enumerate(dve_chunks):
        if _spread(idx, len(dve_chunks), min(n_maxpool, len(dve_chunks))):
            maxpool.add(k)
    TAIL = NCH - 8  # last chunks: DVE-route masks go to Pool (it idles in
    # the drain); ACT-route masks stay on DVE
    mask_pool = [False] * NCH
    for k in range(TAIL, NCH):
        if not act_route[k]:
            mask_pool[k] = True
    n_tailpool = sum(mask_pool)
    dve_head = [k for k in dve_chunks if k < TAIL]
    npool = min(n_maskpool - n_tailpool, len(dve_head))
    for idx, k in enumerate(dve_head):
        if _spread(idx, len(dve_head), npool):
            mask_pool[k] = True
    rem = n_maskpool - n_tailpool - npool
    act_head = [k for k in range(NCH) if act_route[k] and k < TAIL]
    for idx, k in enumerate(act_head):
        if _spread(idx, len(act_head), min(max(rem, 0), len(act_head))):
            mask_pool[k] = True

    with tile.TileContext(nc) as tc, ExitStack() as ctx:
        singles = ctx.enter_context(tc.tile_pool(name="singles", bufs=1))

        rhsW = singles.tile([128, KC * FO2], F32)
        rhsW16 = singles.tile([128, KC * FO2], F16)
        ident32 = singles.tile([128, 128], F32)
        make_identity(nc, ident32)
        identp = singles.tile([128, 128], F16)
        make_identity(nc, identp)
        ones16 = singles.tile([128, 1], F16)
        nc.vector.memset(ones16, 1.0)
        scratch = singles.tile([128, f_out], F32)
        a12b = singles.tile([128, 2 * f_out], F32)
        a1b = a12b[:, 0:f_out]
        a2b = a12b[:, f_out : 2 * f_out]

        h_sb = singles.tile([128, NCH * f_out], F16)
        sstage = singles.tile([128, 2 * NCH], F32)   # per chunk: [s1 s2]
        ustg = singles.tile([128, SUB], F32)         # exp(s1) own rows
        pstg = singles.tile([128, SUB], F32)         # exp(0.2 s1)
        vstg = singles.tile([128, NCH], F32)         # exp(s2) all chunks
        qstg = singles.tile([128, NCH], F32)         # exp(0.2 s2)
        stack3 = singles.tile([128, 3 * SUB], F16)   # [s1 | u | p] fp16
        b3 = singles.tile([128, 3 * rows], F16)      # broadcast rows [s1|u|p]
        s1b = b3[:, 0:rows]
        ub = b3[:, rows : 2 * rows]
        pb = b3[:, 2 * rows : 3 * rows]
        rsTA = singles.tile([128, SUB], F32)
        rsT = singles.tile([128, SUB], F32)
        rinv = singles.tile([128, SUB], F32)
        rsA_sb = singles.tile([1, rows], F32)
        rsB_sb = singles.tile([1, rows], F32)
        RS_SPLIT = 48

        # ---- constants: rhsW = [W | W@a1 | W@a2] per k-chunk ----
        nc.sync.dma_start(a12b, dram3(a_ap, 0, [[0, 128], [1, 2 * f_out]]))
        rhsW3 = rhsW.rearrange("p (k f) -> p k f", k=KC)
        nc.sync.dma_start(
            rhsW3[:, :, 0:f_out],
            dram3(w_ap, 0, [[f_out, 128], [128 * f_out, KC], [1, f_out]]),
        )
        # (NOTE tensor_tensor_reduce crashes the device — use scalar_tensor_tensor)
        for kc in range(KC):
            for ai, ab in ((0, a1b), (1, a2b)):
                nc.vector.scalar_tensor_tensor(
                    out=scratch,
                    in0=rhsW[:, kc * FO2 : kc * FO2 + f_out],
                    scalar=1.0,
                    in1=ab,
                    op0=OP.mult,
                    op1=OP.mult,
                    accum_out=rhsW[:, kc * FO2 + f_out + ai : kc * FO2 + f_out + ai + 1],
                )
        nc.vector.tensor_copy(rhsW16, rhsW)

        acc_pool = ctx.enter_context(tc.tile_pool(name="acc", bufs=1, space="PSUM"))
        acc_ps = [
            acc_pool.tile([128, 512], F32, name=f"acc{ih}", tag=f"acc{ih}")
            for ih in range(IH)
        ]
        rs_pool = ctx.enter_context(tc.tile_pool(name="rsp", bufs=1, space="PSUM"))
        rs_ps = [
            rs_pool.tile([1, 512], F32, name=f"rs{ih}", tag=f"rs{ih}")
            for ih in range(IH)
        ]

        with ExitStack() as mctx:
            xtp = mctx.enter_context(tc.tile_pool(name="xtp", bufs=3))
            hpsp = mctx.enter_context(tc.tile_pool(name="hpsp", bufs=hpsp_bufs, space="PSUM"))
            tsp = mctx.enter_context(tc.tile_pool(name="tsp", bufs=1, space="PSUM"))
            adjp = mctx.enter_context(tc.tile_pool(name="adjp", bufs=adj_bufs))
            ep = mctx.enter_context(tc.tile_pool(name="ep", bufs=e_bufs))
            pp = mctx.enter_context(tc.tile_pool(name="pp", bufs=p_bufs))

            adj_eng = nc.scalar if adj_dma == "act" else nc.sync
            adj_tiles = {}

            def load_adj_batch(b):
                t = adjp.tile([128, adjb * rows], F16, tag="adj", name=f"adj_{b}")
                adj_eng.dma_start(
                    t,
                    dram3(
                        adjt_ap,
                        b * adjb * 128 * rows,
                        [[rows, 128], [128 * rows, adjb], [1, rows]],
                    ),
                )
                adj_tiles[b] = t

            # (adjT prefetch is interleaved after the first xT group loads)

            def emit_attn(c):
                b = c // adjb
                if b not in adj_tiles:
                    load_adj_batch(b)
                adj_sl = adj_tiles[b][:, (c % adjb) * rows : (c % adjb + 1) * rows]

                if act_route[c]:
                    lr = ep.tile([128, rows], F16, tag="lr")
                    nc.scalar.activation(
                        out=lr, in_=s1b, func=AF.Prelu,
                        bias=sstage[:, 2 * c + 1 : 2 * c + 2],
                        scale=1.0, alpha=NEG_SLOPE,
                    )
                    e_t = ep.tile([128, rows], F16, tag="e")
                    nc.scalar.activation(out=e_t, in_=lr, func=AF.Exp)
                else:
                    a_t = ep.tile([128, rows], F16, tag="lr")
                    nc.vector.tensor_scalar(
                        out=a_t, in0=ub, scalar1=vstg[:, c : c + 1], scalar2=None,
                        op0=OP.mult, op1=OP.bypass,
                    )
                    b_t = ep.tile([128, rows], F16, tag="bt")
                    nc.vector.tensor_scalar(
                        out=b_t, in0=pb, scalar1=qstg[:, c : c + 1], scalar2=None,
                        op0=OP.mult, op1=OP.bypass,
                    )
                    e_t = ep.tile([128, rows], F16, tag="e")
                    meng = nc.gpsimd if c in maxpool else nc.vector
                    meng.tensor_tensor(out=e_t, in0=a_t, in1=b_t, op=OP.max)

                p_t = pp.tile([128, rows], F16, tag="p")
                mask_eng = nc.gpsimd if mask_pool[c] else nc.vector
                mask_eng.tensor_tensor(out=p_t, in0=e_t, in1=adj_sl, op=OP.mult)

                for ih in range(IH):
                    nc.tensor.matmul(
                        acc_ps[ih],
                        lhsT=h_sb[:, c * f_out : (c + 1) * f_out],
                        rhs=p_t[:, ih * 512 : (ih + 1) * 512],
                        start=(c == 0),
                        stop=(c == NCH - 1),
                        skip_group_check=True,
                    )
                for ih in range(IH):
                    nc.tensor.matmul(
                        rs_ps[ih],
                        lhsT=ones16,
                        rhs=p_t[:, ih * 512 : (ih + 1) * 512],
                        start=(c in (0, RS_SPLIT)),
                        stop=(c in (RS_SPLIT - 1, NCH - 1)),
                        skip_group_check=True,
                    )
                if c == RS_SPLIT - 1:
                    # flush the first rowsum group early so its DRAM
                    # reshape roundtrip overlaps the remaining chunks
                    for ih in range(IH):
                        nc.vector.tensor_copy(
                            rsA_sb[:, ih * 512 : (ih + 1) * 512], rs_ps[ih]
                        )
                    nc.sync.dma_start(
                        dram3(rsd, 0, [[0, 1], [1, rows]]), rsA_sb
                    )
                    nc.sync.dma_start(
                        rsTA, dram3(rsd, 0, [[1, 128], [128, SUB]])
                    )

            # ---- phase H: h chunks + s1/s2 staging (own chunks = 0..7),
            # attention emission interleaved with a lag so the whole
            # pipeline (E -> mask -> matmul) starts as soon as the first
            # chunks and broadcasts are ready ----
            LAG = lag
            HB = 2  # chunks per hps PSUM tile; copies batched per tile
            hcopy_k = 0
            xts = None
            for cc in range(NCH):
                g = cc // xg
                if cc % xg == 0:
                    xts = []
                    for kc in range(KC):
                        xt = xtp.tile([128, xg * 128], F16, tag=f"xt{kc}")
                        nc.sync.dma_start(
                            xt, xt_ap[kc * 128 : (kc + 1) * 128,
                                      g * xg * 128 : (g + 1) * xg * 128]
                        )
                        xts.append(xt)
                    if g < 2 and g not in adj_tiles:
                        load_adj_batch(g)
                k = cc % HB
                if k == 0:
                    hps = hpsp.tile([128, HB * FO2], F32, tag="hps")
                # W-part and s-part as separate accumulation groups so the
                # h columns don't wait on the w1/w2 setup chain
                for kc in range(KC):
                    nc.tensor.matmul(
                        hps[:, k * FO2 : k * FO2 + f_out],
                        lhsT=xts[kc][:, (cc % xg) * 128 : (cc % xg + 1) * 128],
                        rhs=rhsW16[:, kc * FO2 : kc * FO2 + f_out],
                        start=(kc == 0),
                        stop=(kc == KC - 1),
                        skip_group_check=True,
                    )
                for kc in range(KC):
                    nc.tensor.matmul(
                        hps[:, k * FO2 + f_out : (k + 1) * FO2],
                        lhsT=xts[kc][:, (cc % xg) * 128 : (cc % xg + 1) * 128],
                        rhs=rhsW16[:, kc * FO2 + f_out : (kc + 1) * FO2],
                        start=(kc == 0),
                        stop=(kc == KC - 1),
                        skip_group_check=True,
                    )
                if k == HB - 1:
                    c0 = cc - k
                    h3 = hps.rearrange("p (b f) -> p b f", b=HB)
                    heng_v = h_copy_cycle[hcopy_k % len(h_copy_cycle)] == "v"
                    hcopy_k += 1
                    if heng_v:
                        nc.vector.tensor_copy(
                            h_sb[:, c0 * f_out : (cc + 1) * f_out],
                            h3[:, :, 0:f_out],
                        )
                        nc.vector.tensor_copy(
                            sstage[:, 2 * c0 : 2 * (cc + 1)], h3[:, :, f_out:FO2]
                        )
                    else:
                        nc.scalar.copy(
                            h_sb[:, c0 * f_out : (cc + 1) * f_out],
                            h3[:, :, 0:f_out],
                        )
                        nc.scalar.copy(
                            sstage[:, 2 * c0 : 2 * (cc + 1)], h3[:, :, f_out:FO2]
                        )
                if cc % xg == xg - 1:
                    # v/q scalars for this group's chunks (DVE route inputs)
                    s3 = sstage.rearrange("p (c two) -> p two c", two=2)
                    nc.scalar.activation(
                        out=vstg[:, g * xg : (g + 1) * xg],
                        in_=s3[:, 1, g * xg : (g + 1) * xg], func=AF.Exp)
                    nc.scalar.activation(
                        out=qstg[:, g * xg : (g + 1) * xg],
                        in_=s3[:, 1, g * xg : (g + 1) * xg], func=AF.Exp,
                        scale=NEG_SLOPE)
                if cc == xg - 1:
                    # own chunks done -> u/p/s1 broadcast setup
                    s3 = sstage.rearrange("p (c two) -> p two c", two=2)
                    own_s1 = s3[:, 0, 0:SUB]
                    nc.scalar.activation(out=ustg, in_=own_s1, func=AF.Exp)
                    nc.scalar.activation(out=pstg, in_=own_s1, func=AF.Exp,
                                         scale=NEG_SLOPE)
                    nc.vector.tensor_copy(stack3[:, 0:SUB], own_s1)
                    nc.vector.tensor_copy(stack3[:, SUB : 2 * SUB], ustg)
                    nc.vector.tensor_copy(stack3[:, 2 * SUB : 3 * SUB], pstg)
                    t3 = tsp.tile([3 * SUB, 128], F16, tag="t3")
                    nc.tensor.transpose(t3, stack3, identp)
                    t3s = xtp.tile([3 * SUB, 128], F16, tag="t3s")
                    nc.vector.tensor_copy(t3s, t3)
                    # SP-queue position here is early (before xt g1 / adjT b1),
                    # so these small DMAs aren't stuck behind bulk loads in
                    # the DMA-device queue
                    nc.sync.dma_start(
                        dram3(bvec, 0, [[128, 3 * SUB], [1, 128]]), t3s
                    )
                    nc.sync.dma_start(
                        b3, dram3(bvec, 0, [[0, 128], [1, 3 * rows]])
                    )
                if cc >= xg + LAG:
                    emit_attn(cc - xg - LAG)

            # ---- phase A: drain remaining attention chunks ----
            for c in range(max(0, NCH - xg - LAG), NCH):
                emit_attn(c)

        # ---- phase F: normalize + elu + store ----
        with ExitStack() as fctx:
            fpool = fctx.enter_context(tc.tile_pool(name="fpool", bufs=4))
            fps = fctx.enter_context(tc.tile_pool(name="fps", bufs=4, space="PSUM"))
            # flush the second rowsum group and reshape via DRAM
            for ih in range(IH):
                nc.vector.tensor_copy(
                    rsB_sb[:, ih * 512 : (ih + 1) * 512], rs_ps[ih]
                )
            nc.sync.dma_start(dram3(rsd, rows, [[0, 1], [1, rows]]), rsB_sb)
            nc.sync.dma_start(rsT, dram3(rsd, rows, [[1, 128], [128, SUB]]))
            nc.vector.tensor_tensor(out=rsT, in0=rsT, in1=rsTA, op=OP.add)
            nc.vector.reciprocal(rinv, rsT)
            # transposes + relu parts run while the rowsum roundtrip is in
            # flight (mostly on the by-now idle ACT engine); only the rinv
            # applications wait for it.
            rinvn = fpool.tile([128, SUB], F32, bufs=1)
            nc.vector.tensor_scalar(
                out=rinvn, in0=rinv, scalar1=-1.0, scalar2=None,
                op0=OP.mult, op1=OP.bypass,
            )
            hTn = fpool.tile([128, rows], F32, tag="hTn", bufs=1)
            for ih in range(IH):
                nc.vector.tensor_copy(hTn[:, ih * 512 : (ih + 1) * 512], acc_ps[ih])
            relus, nrelus = [], []
            for s in range(SUB):
                tb = fps.tile([128, 128], F32)
                nc.tensor.transpose(tb, hTn[:, s * 128 : (s + 1) * 128], ident32)
                rl = fpool.tile([128, f_out], F32, tag=f"rl{s}", bufs=1)
                nc.scalar.activation(out=rl, in_=tb, func=AF.Relu)
                nr = fpool.tile([128, f_out], F32, tag=f"nr{s}", bufs=1)
                nc.scalar.activation(out=nr, in_=tb, func=AF.Relu, scale=-1.0)
                relus.append(rl)
                nrelus.append(nr)
            obuf = fpool.tile([128, SUB * f_out], F32, bufs=1)
            for s in range(SUB):
                # elu(v), v = h' * rinv: relu(tb)*rinv - 1 + exp(-rinv*relu(-tb))
                t3 = fpool.tile([128, f_out], F32)
                nc.scalar.activation(out=t3, in_=nrelus[s], func=AF.Exp,
                                     scale=rinvn[:, s : s + 1])
                t1 = fpool.tile([128, f_out], F32)
                nc.vector.tensor_scalar(
                    out=t1, in0=relus[s], scalar1=rinv[:, s : s + 1], scalar2=None,
                    op0=OP.mult, op1=OP.bypass,
                )
                nc.vector.scalar_tensor_tensor(
                    out=obuf[:, s * f_out : (s + 1) * f_out], in0=t3,
                    scalar=-1.0, in1=t1, op0=OP.add, op1=OP.add,
                )
            ob3 = obuf.rearrange("p (s f) -> p s f", s=SUB)
            nc.sync.dma_start(
                dram3(out_ap, 0, [[f_out, 128], [128 * f_out, SUB], [1, f_out]]),
                ob3,
            )

    nc.compile()
    return nc


_CACHE = {}


def _compiled_full():
    if "nc" not in _CACHE:
        _CACHE["nc"] = build_gat()
    return _CACHE["nc"]


def _prep_in_maps(x, W, a, adj):
    """Host-side sharding/layout prep: per-core transposed fp16 views with
    this core's own j-rows permuted first."""
    rows = N_FULL // N_CORES
    xT16 = np.ascontiguousarray(x.T.astype(np.float16))      # [f_in, n]
    W = np.ascontiguousarray(W.astype(np.float32))
    a = np.ascontiguousarray(a.astype(np.float32))
    in_maps = []
    for c in range(N_CORES):
        sl = slice(c * rows, (c + 1) * rows)
        xt_c = np.concatenate(
            [xT16[:, sl], xT16[:, : c * rows], xT16[:, (c + 1) * rows :]], axis=1
        )
        adjT = adj[sl].astype(np.float16).T                   # [n, rows]
        adjt_c = np.concatenate(
            [adjT[sl], adjT[: c * rows], adjT[(c + 1) * rows :]], axis=0
        )
        in_maps.append(
            {
                "xt": np.ascontiguousarray(xt_c),
                "adjt": np.ascontiguousarray(adjt_c),
                "w": W,
                "a": a,
            }
        )
    return in_maps


def kernel(x, W, a, adj):
    from concourse.bass_utils import run_bass_kernel_spmd

    nc = _compiled_full()
    x = np.asarray(x, dtype=np.float32)
    W = np.asarray(W, dtype=np.float32)
    a = np.asarray(a, dtype=np.float32)
    adj = np.asarray(adj)
    in_maps = _prep_in_maps(x, W, a, adj)
    res = run_bass_kernel_spmd(nc, in_maps, core_ids=list(range(N_CORES)))
    out = np.concatenate([res.results[c]["out"] for c in range(N_CORES)], axis=0)
    return out.astype(np.float32)
